# revision 7
# baseline (speedup 1.0000x reference)
"""Transformer block v2: bf16 operands, xbar transposes, resident FC weights.

Sharding (unchanged): core i owns tokens [512i,512(i+1)) for LN/MLP/residual,
heads {2i,2i+1} for attention. Collectives: 2x chunked AllGather of LN1(x)^T
(bf16, 2x512KB in / 4MB out each), AllToAll of y^T (bf16 1MB).

vs v1: all matmul operands bf16 (FWL makes LDWEIGHTS ~13ns), FC weights
prefetched to SBUF during attention, LN gamma/beta folded into w_attn/w_fc on
host, k-bias dropped (softmax-invariant), v-bias folded into proj bias,
softmax normalize via single tensor_tensor divide (DVE reciprocal was 3.4us),
LN transposes via DMA xbar instead of PE+DVE.
"""
import os
import numpy as np
import ml_dtypes
from contextlib import ExitStack

import concourse.bass as bass
import concourse.bacc as bacc
import concourse.tile as tile
from concourse import mybir
from concourse.bass_utils import run_bass_kernel_spmd

P = 128
B, T, C = 2, 2048, 1024
H, D = 16, 64
FF = 4 * C
NCORE = 8
TLOC = (B * T) // NCORE          # 512
NT = TLOC // P                   # 4
NC8 = C // P                     # 8
NF = FF // P                     # 32
EPS = 1e-5
F32 = mybir.dt.float32
BF16 = mybir.dt.bfloat16
AF = mybir.ActivationFunctionType
ALU = mybir.AluOpType
NPBF = ml_dtypes.bfloat16

_CACHE = {}


def build_nc():
    nc = bacc.Bacc("TRN2", num_devices=NCORE)

    dp = nc.declare_dram_parameter
    x_loc = dp("x_loc", [TLOC, C], F32, isOutput=False)
    wq = dp("wq", [C, P], BF16, isOutput=False)
    wk = dp("wk", [C, P], BF16, isOutput=False)
    wv = dp("wv", [C, P], BF16, isOutput=False)
    bq = dp("bq", [P, 1], F32, isOutput=False)
    w_proj = dp("w_proj", [C, C], BF16, isOutput=False)
    w_fc = dp("w_fc", [C, FF], BF16, isOutput=False)
    b_fc = dp("b_fc", [FF, 1], F32, isOutput=False)
    w_proj2 = dp("w_proj2", [FF, C], BF16, isOutput=False)
    bias2 = dp("bias2", [2, C], F32, isOutput=False)
    identb = dp("identb", [P, P], BF16, isOutput=False)
    onesv = dp("onesv", [P, D], BF16, isOutput=False)
    out_loc = dp("out_loc", [TLOC, C], BF16, isOutput=True)

    ag_in = nc.dram_tensor("ag_in", [C, TLOC], BF16)
    ag_out = nc.dram_tensor("ag_out", [NCORE, C, TLOC], BF16,
                            addr_space="Shared")
    a2a_in = nc.dram_tensor("a2a_in", [NCORE, P, TLOC], BF16)
    a2a_out = nc.dram_tensor("a2a_out", [NCORE, P, TLOC], BF16)

    with tile.TileContext(nc) as tc, ExitStack() as ctx:
        # ---------------- constants ----------------
        cst = ctx.enter_context(tc.tile_pool(name="const", bufs=1))
        eps_t = cst.tile([P, 1], F32, tag="eps", name="eps")
        nc.vector.memset(eps_t[:], EPS)
        bq_t = cst.tile([P, 1], F32, tag="bq", name="bq")
        nc.sync.dma_start(bq_t[:], bq[:])
        idb = cst.tile([P, P], BF16, tag="idb", name="idb")
        nc.sync.dma_start(idb[:], identb[:])
        bproj_bc = cst.tile([P, C], F32, tag="bproj", name="bproj")
        nc.sync.dma_start(bproj_bc[:],
                          bass.AP(tensor=bias2, offset=0, ap=[[0, P], [1, C]]))
        bproj2_bc = cst.tile([P, C], F32, tag="bproj2", name="bproj2")
        nc.sync.dma_start(bproj2_bc[:],
                          bass.AP(tensor=bias2, offset=C, ap=[[0, P], [1, C]]))

        # Persistent pools (LIFO close order):
        # abig,wqkv close after s4; wfc/wpj open after that, close after
        # s7/s5; mlp (h2T, out1, gT) and cst close at the end.
        mlp_cm = tc.tile_pool(name="mlp", bufs=1)
        mlp = mlp_cm.__enter__()
        abig_cm = tc.tile_pool(name="abig", bufs=1)
        abig = abig_cm.__enter__()
        wqkv_cm = tc.tile_pool(name="wqkv", bufs=1)
        wqkvp = wqkv_cm.__enter__()

        # ============ s1: LN1 -> bf16, PE transpose, ag input ============
        with tc.tile_pool(name="ph1", bufs=2) as ph1, \
             tc.tile_pool(name="ph1s", bufs=4) as ph1s, \
             tc.tile_pool(name="ph1p", bufs=4, space="PSUM") as ph1p, \
             tc.tile_pool(name="ph1o", bufs=1) as ph1o, \
             nc.named_scope("s1_ln1"):
            xnT = [ph1o.tile([P, TLOC], BF16, tag=f"xnT{cc}", name=f"xnT{cc}")
                   for cc in range(NC8)]
            for tt in range(NT):
                xt = ph1.tile([P, C], F32, tag="x", name="x")
                nc.sync.dma_start(xt[:], x_loc[tt * P:(tt + 1) * P, :])
                st = ph1s.tile([P, 2, 6], F32, tag="st", name="st")
                nc.vector.bn_stats(st[:, 0, :], xt[:, 0:512])
                nc.vector.bn_stats(st[:, 1, :], xt[:, 512:1024])
                mv = ph1s.tile([P, 2], F32, tag="mv", name="mv")
                nc.vector.bn_aggr(mv[:], st[:])
                sq = ph1s.tile([P, 1], F32, tag="sq", name="sq")
                nc.scalar.activation(sq[:], mv[:, 1:2], AF.Sqrt, bias=eps_t[:])
                rstd = ph1s.tile([P, 1], F32, tag="rstd", name="rstd")
                nc.vector.reciprocal(rstd[:], sq[:])
                xn = ph1.tile([P, C], BF16, tag="xn", name="xn")
                nc.vector.tensor_scalar(xn[:], xt[:], mv[:, 0:1], rstd[:],
                                        ALU.subtract, ALU.mult)
                for cc in range(NC8):
                    tp1 = ph1p.tile([P, P], BF16, tag="tp1", name="tp1")
                    nc.tensor.transpose(tp1[:], xn[:, cc * P:(cc + 1) * P], idb[:])
                    nc.vector.tensor_copy(xnT[cc][:, tt * P:(tt + 1) * P], tp1[:])
            for cc in range(NC8):
                nc.sync.dma_start(ag_in[cc * P:(cc + 1) * P, :], xnT[cc][:])

        with nc.named_scope("s1b_ag"):
            nc.gpsimd.collective_compute(
                "AllGather", ALU.bypass, ins=[ag_in[:]], outs=[ag_out[:]],
                replica_groups=[list(range(NCORE))])

        # qkv weights (needed immediately in s2)
        wq_t = wqkvp.tile([P, NC8, P], BF16, tag="wq", name="wq_t")
        wk_t = wqkvp.tile([P, NC8, P], BF16, tag="wk", name="wk_t")
        wv_t = wqkvp.tile([P, NC8, P], BF16, tag="wv", name="wv_t")
        for cc in range(NC8):
            nc.sync.dma_start(wq_t[:, cc, :], wq[cc * P:(cc + 1) * P, :])
            nc.sync.dma_start(wk_t[:, cc, :], wk[cc * P:(cc + 1) * P, :])
            nc.sync.dma_start(wv_t[:, cc, :], wv[cc * P:(cc + 1) * P, :])

        # ============ s2: qkv matmuls ============
        qTz = [[abig.tile([P, TLOC], BF16, tag=f"qz{hl}_{i}", name=f"qz{hl}_{i}")
                for i in range(NCORE)] for hl in range(2)]
        kTz = [[abig.tile([P, TLOC], BF16, tag=f"kz{hl}_{i}", name=f"kz{hl}_{i}")
                for i in range(NCORE)] for hl in range(2)]
        vTt = [abig.tile([P, TLOC], BF16, tag=f"vT{i}", name=f"vT{i}")
               for i in range(NCORE)]
        # rows D:P stay zero so score matmuls contract over a full 128
        # partitions (64 real + 64 zero) -- full-rate SBUF streaming.
        for hl in range(2):
            for i in range(NCORE):
                nc.gpsimd.memset(qTz[hl][i][D:P, :], 0.0)
                nc.gpsimd.memset(kTz[hl][i][D:P, :], 0.0)
        yTt = [abig.tile([P, TLOC], BF16, tag=f"yT{i}", name=f"yT{i}")
               for i in range(NCORE)]
        vo_b = [abig.tile([P, T // P, 2, P], BF16, tag=f"vo{b}", name=f"vo{b}")
                for b in range(B)]

        ph2_cm = [tc.tile_pool(name="ph2h", bufs=16),
                  tc.tile_pool(name="ph2p", bufs=4, space="PSUM"),
                  tc.tile_pool(name="ph3p", bufs=2, space="PSUM")]
        ph2h, ph2p, ph3p = [c.__enter__() for c in ph2_cm]
        with nc.named_scope("s2_qkv"):
            for t8 in range(NCORE):
                hx = []
                for cc in range(NC8):
                    h_ = ph2h.tile([P, TLOC], BF16, tag="hx", name="hx")
                    nc.sync.dma_start(h_[:], ag_out[t8, cc * P:(cc + 1) * P, :])
                    hx.append(h_)
                for wt, kind in ((wq_t, "q"), (wk_t, "k"), (wv_t, "v")):
                    ps = ph2p.tile([P, TLOC], F32, tag="ps2", name="ps2")
                    for cc in range(NC8):
                        nc.tensor.matmul(ps[:], wt[:, cc, :], hx[cc][:],
                                         start=(cc == 0), stop=(cc == NC8 - 1))
                    if kind == "q":
                        for hl in range(2):
                            nc.vector.tensor_scalar_add(
                                qTz[hl][t8][0:D, :],
                                ps[hl * D:(hl + 1) * D, :],
                                bq_t[hl * D:(hl + 1) * D, :])
                    elif kind == "k":
                        for hl in range(2):
                            nc.vector.tensor_copy(kTz[hl][t8][0:D, :],
                                                  ps[hl * D:(hl + 1) * D, :])
                    else:
                        nc.vector.tensor_copy(vTt[t8][:], ps[:])

        # ============ s3: V -> token-major vo (PE transpose) ============
        with nc.named_scope("s3_vtr"):
            ones_src = bass.AP(tensor=onesv, offset=0,
                               ap=[[D, P], [0, T // P], [1, D]])
            for b in range(B):
                for hl in range(2):
                    nc.sync.dma_start(vo_b[b][:, :, hl, D:P], ones_src)
                for kt in range(T // P):
                    t8 = b * NT + kt // NT
                    koff = (kt % NT) * P
                    tp = ph3p.tile([P, P], BF16, tag="vtp", name="vtp")
                    nc.tensor.transpose(tp[:], vTt[t8][:, koff:koff + P], idb[:])
                    nc.vector.tensor_copy(
                        vo_b[b][:, kt, :, 0:D],
                        tp[:].rearrange("p (h d) -> p h d", h=2))
        for c in reversed(ph2_cm):
            c.__exit__(None, None, None)

        # ============ s4: attention ============
        # Both heads per kt emitted together with PV lagging one kt, so the
        # Tensor queue has 4-matmul runs and ACT exp latency stays hidden.
        with tc.tile_pool(name="ptp", bufs=8) as ptp, \
             tc.tile_pool(name="spsum", bufs=2, space="PSUM") as spsum, \
             tc.tile_pool(name="ypsum", bufs=2, space="PSUM") as ypsum, \
             nc.named_scope("s4_attn"):
            for b in range(B):
                for qc in range(NT - 1, -1, -1):
                    q8 = b * NT + qc
                    nkt = NT * (qc + 1)
                    yps = [ypsum.tile([P, TLOC], F32, tag=f"yps{hl}",
                                      name=f"yps{hl}") for hl in range(2)]
                    pend = []  # (kt, hl, pt) with PV not yet emitted
                    for kt in range(nkt):
                        t8k = b * NT + kt // NT
                        koff = (kt % NT) * P
                        pts = []
                        for hl in range(2):
                            sps = spsum.tile([P, TLOC], F32, tag=f"sps{hl}",
                                             name=f"sps{hl}")
                            nc.tensor.matmul(sps[:],
                                             kTz[hl][t8k][:, koff:koff + P],
                                             qTz[hl][q8][:], start=True, stop=True)
                            pt = ptp.tile([P, TLOC], BF16, tag=f"pt{hl}",
                                          name=f"pt{hl}")
                            nc.scalar.activation(pt[:], sps[:], AF.Exp, scale=0.125)
                            m = kt - NT * qc
                            if m >= 0:
                                nc.gpsimd.affine_select(
                                    pt[:], pt[:], pattern=[[1, TLOC]],
                                    compare_op=ALU.is_ge, fill=0.0,
                                    base=-P * m, channel_multiplier=-1)
                            pts.append(pt)
                        for pkt, phl, ppt in pend:
                            nc.tensor.matmul(yps[phl][:],
                                             vo_b[b][:, pkt, phl, :], ppt[:],
                                             start=(pkt == 0), stop=False)
                        pend = [(kt, 0, pts[0]), (kt, 1, pts[1])]
                    for pkt, phl, ppt in pend:
                        nc.tensor.matmul(yps[phl][:], vo_b[b][:, pkt, phl, :],
                                         ppt[:], start=(pkt == 0), stop=True)
                    for hl in range(2):
                        hs = slice(hl * D, (hl + 1) * D)
                        rec = ptp.tile([D, TLOC], F32, tag="rec", name="rec")
                        if b == 1 and qc <= 1:
                            # tail: ACT is idle; 1/s = exp(-ln(s)) unblocks
                            # the a2a sooner than DVE's 3.4us reciprocal
                            lns = ptp.tile([D, TLOC], F32, tag="lns", name="lns")
                            nc.scalar.activation(lns[:], yps[hl][D:P, :], AF.Ln)
                            nc.scalar.activation(rec[:], lns[:], AF.Exp,
                                                 scale=-1.0)
                        else:
                            nc.vector.reciprocal(rec[:], yps[hl][D:P, :])
                        nc.vector.tensor_mul(yTt[q8][hs, :], yps[hl][0:D, :],
                                             rec[:])
                    nc.sync.dma_start(a2a_in[q8], yTt[q8][:])

        wqkv_cm.__exit__(None, None, None)
        abig_cm.__exit__(None, None, None)
        with nc.named_scope("s4b_a2a"):
            nc.gpsimd.collective_compute(
                "AllToAll", ALU.bypass, ins=[a2a_in[:]], outs=[a2a_out[:]],
                replica_groups=[list(range(NCORE))])

        wfc_cm = tc.tile_pool(name="wfc", bufs=1)
        wfcp = wfc_cm.__enter__()
        wpj_cm = tc.tile_pool(name="wpj", bufs=1)
        wpjp = wpj_cm.__enter__()
        wfc_sb = wfcp.tile([P, NC8, FF], BF16, tag="wfc", name="wfc_sb")
        for cc in range(NC8):
            nc.sync.dma_start(wfc_sb[:, cc, :], w_fc[cc * P:(cc + 1) * P, :])
        wproj_sb = wpjp.tile([P, NC8, 2, TLOC], BF16, tag="wpj", name="wproj_sb")
        for r8 in range(NC8):
            nc.sync.dma_start(
                wproj_sb[:, r8, :, :],
                w_proj[r8 * P:(r8 + 1) * P, :].rearrange("p (l n) -> p l n", l=2))

        # long-lived activations: out1 (s5..s8), h2T (s6..s7), gT (s7..s8)
        out1 = [mlp.tile([P, C], F32, tag=f"o1_{tt}", name=f"o1_{tt}")
                for tt in range(NT)]
        h2T = [mlp.tile([P, TLOC], BF16, tag=f"h2T{cc}", name=f"h2T{cc}")
               for cc in range(NC8)]
        gT = [mlp.tile([P, TLOC], BF16, tag=f"gT{fb}", name=f"gT{fb}")
              for fb in range(NF)]

        # ============ s5: proj + residual ============
        ph5_cm = [tc.tile_pool(name="ph5y", bufs=1),
                  tc.tile_pool(name="ph5p", bufs=3, space="PSUM")]
        ph5y, ph5p = [c.__enter__() for c in ph5_cm]
        st6a = [mlp.tile([P, 2, 6], F32, tag=f"st6a{tt}", name=f"st6a{tt}")
                for tt in range(NT)]
        with nc.named_scope("s5_proj"):
            yf = [ph5y.tile([P, TLOC], BF16, tag=f"yf{r8}", name=f"yf{r8}")
                  for r8 in range(NCORE)]
            for r8 in range(NCORE):
                nc.sync.dma_start(yf[r8][:], a2a_out[r8])
            for tt in range(NT):
                xt2 = ph5y.tile([P, C], F32, tag="x2", name="x2")
                nc.sync.dma_start(xt2[:], x_loc[tt * P:(tt + 1) * P, :])
                for cl in range(2):
                    ps = ph5p.tile([P, TLOC], F32, tag="ps5", name="ps5")
                    for r8 in range(NC8):
                        nc.tensor.matmul(ps[:], yf[r8][:, tt * P:(tt + 1) * P],
                                         wproj_sb[:, r8, cl, :],
                                         start=(r8 == 0), stop=(r8 == NC8 - 1))
                    o1s = out1[tt][:, cl * TLOC:(cl + 1) * TLOC]
                    nc.vector.tensor_add(o1s, ps[:],
                                         xt2[:, cl * TLOC:(cl + 1) * TLOC])
                    nc.vector.tensor_add(o1s, o1s,
                                         bproj_bc[:, cl * TLOC:(cl + 1) * TLOC])
                    nc.vector.bn_stats(st6a[tt][:, cl, :], o1s)
        for c in reversed(ph5_cm):
            c.__exit__(None, None, None)
        wpj_cm.__exit__(None, None, None)

        # ============ s6: LN2 -> bf16, PE transpose ============
        with tc.tile_pool(name="ph6", bufs=2) as ph6, \
             tc.tile_pool(name="ph6s", bufs=4) as ph6s, \
             tc.tile_pool(name="ph6p", bufs=4, space="PSUM") as ph6p, \
             nc.named_scope("s6_ln2"):
            for tt in range(NT):
                ot = out1[tt]
                mv = ph6s.tile([P, 2], F32, tag="mv6", name="mv6")
                nc.vector.bn_aggr(mv[:], st6a[tt][:])
                sq = ph6s.tile([P, 1], F32, tag="sq6", name="sq6")
                nc.scalar.activation(sq[:], mv[:, 1:2], AF.Sqrt, bias=eps_t[:])
                rstd = ph6s.tile([P, 1], F32, tag="rstd6", name="rstd6")
                nc.vector.reciprocal(rstd[:], sq[:])
                h2n = ph6.tile([P, C], BF16, tag="h2n", name="h2n")
                nc.vector.tensor_scalar(h2n[:], ot[:], mv[:, 0:1], rstd[:],
                                        ALU.subtract, ALU.mult)
                nc.vector.tensor_add(ot[:], ot[:], bproj2_bc[:])
                for cc in range(NC8):
                    tp6 = ph6p.tile([P, P], BF16, tag="tp6", name="tp6")
                    nc.tensor.transpose(tp6[:], h2n[:, cc * P:(cc + 1) * P], idb[:])
                    nc.vector.tensor_copy(h2T[cc][:, tt * P:(tt + 1) * P], tp6[:])

        # ============ s7: fc + gelu ============
        with tc.tile_pool(name="fcb", bufs=4) as fcb, \
             tc.tile_pool(name="fcp", bufs=3, space="PSUM") as fcp, \
             nc.named_scope("s7_fc"):
            for fb in range(NF):
                bt = fcb.tile([P, 1], F32, tag="bfc", name="bfc")
                nc.sync.dma_start(bt[:], b_fc[fb * P:(fb + 1) * P, :])
                ps = fcp.tile([P, TLOC], F32, tag="ps7", name="ps7")
                for cc in range(NC8):
                    nc.tensor.matmul(ps[:], wfc_sb[:, cc, fb * P:(fb + 1) * P],
                                     h2T[cc][:], start=(cc == 0),
                                     stop=(cc == NC8 - 1))
                nc.scalar.activation(gT[fb][:], ps[:], AF.Gelu_apprx_tanh,
                                     bias=bt[:])
        wfc_cm.__exit__(None, None, None)

        # ============ s8: proj2 + residual + out ============
        with tc.tile_pool(name="p2w", bufs=6) as p2w, \
             tc.tile_pool(name="p2p", bufs=1, space="PSUM") as p2p, \
             tc.tile_pool(name="p2o", bufs=2) as p2o, \
             nc.named_scope("s8_proj2"):
            ps2 = {}
            for tt in range(NT):
                for cl in range(2):
                    ps2[(tt, cl)] = p2p.tile([P, TLOC], F32, tag=f"ps2_{tt}_{cl}",
                                             name=f"ps2_{tt}_{cl}")
            for fb in range(NF):
                w2 = p2w.tile([P, 2, TLOC], BF16, tag="w2", name="w2")
                nc.sync.dma_start(
                    w2[:], w_proj2[fb * P:(fb + 1) * P, :].rearrange(
                        "p (l n) -> p l n", l=2))
                for tt in range(NT):
                    for cl in range(2):
                        nc.tensor.matmul(ps2[(tt, cl)][:],
                                         gT[fb][:, tt * P:(tt + 1) * P],
                                         w2[:, cl, :],
                                         start=(fb == 0), stop=(fb == NF - 1))
            for tt in range(NT):
                fin = p2o.tile([P, C], BF16, tag="fin", name="fin")
                for cl in range(2):
                    fs = fin[:, cl * TLOC:(cl + 1) * TLOC]
                    nc.vector.tensor_add(fs, ps2[(tt, cl)][:],
                                         out1[tt][:, cl * TLOC:(cl + 1) * TLOC])
                nc.sync.dma_start(out_loc[tt * P:(tt + 1) * P, :], fin[:])
        mlp_cm.__exit__(None, None, None)

    nc.compile()
    return nc


def _host_inputs(inputs):
    f32 = lambda k: np.asarray(inputs[k], np.float32)
    x = np.ascontiguousarray(f32("x")).reshape(B * T, C)
    ln1_g, ln1_b = f32("ln1_g"), f32("ln1_b")
    ln2_g, ln2_b = f32("ln2_g"), f32("ln2_b")
    w_attn, b_attn = f32("w_attn"), f32("b_attn")
    w_proj, b_proj = f32("w_proj"), f32("b_proj")
    w_fc, b_fc = f32("w_fc"), f32("b_fc")
    w_proj2, b_proj2 = f32("w_proj2"), f32("b_proj2")

    w_attn_eff = w_attn * ln1_g[:, None]
    b_qkv = ln1_b @ w_attn + b_attn
    wq_full = w_attn_eff[:, 0:C]
    wk_full = w_attn_eff[:, C:2 * C]
    wv_full = w_attn_eff[:, 2 * C:3 * C]
    bq_full = b_qkv[0:C]
    bv_full = b_qkv[2 * C:3 * C]
    # k-bias is softmax-invariant (adds a per-query constant); drop it.
    bias_proj_row = b_proj + bv_full @ w_proj
    w_fc_eff = w_fc * ln2_g[:, None]
    b_fc_eff = ln2_b @ w_fc + b_fc

    bias2 = np.stack([bias_proj_row, b_proj2])
    identb = np.eye(P, dtype=NPBF)
    onesv = np.ones((P, D), NPBF)

    wproj_b = w_proj.astype(NPBF)
    wfc_b = w_fc_eff.astype(NPBF)
    wp2_b = w_proj2.astype(NPBF)
    bfc_col = np.ascontiguousarray(b_fc_eff.reshape(FF, 1))

    in_maps = []
    for i in range(NCORE):
        hsl = slice(P * i, P * (i + 1))
        in_maps.append({
            "x_loc": np.ascontiguousarray(x[TLOC * i:TLOC * (i + 1)]),
            "wq": np.ascontiguousarray(wq_full[:, hsl]).astype(NPBF),
            "wk": np.ascontiguousarray(wk_full[:, hsl]).astype(NPBF),
            "wv": np.ascontiguousarray(wv_full[:, hsl]).astype(NPBF),
            "bq": np.ascontiguousarray(bq_full[hsl]).reshape(P, 1),
            "w_proj": wproj_b,
            "w_fc": wfc_b,
            "b_fc": bfc_col,
            "w_proj2": wp2_b,
            "bias2": bias2,
            "identb": identb,
            "onesv": onesv,
        })
    return in_maps


def kernel(**inputs) -> np.ndarray:
    if "nc" not in _CACHE:
        _CACHE["nc"] = build_nc()
    nc = _CACHE["nc"]
    in_maps = _host_inputs(inputs)
    trace = bool(os.environ.get("KERNEL_TRACE"))
    tkw = {}
    if trace:
        tkw["trace"] = True
        if os.environ.get("KERNEL_TRACE_DIR"):
            tkw["tmpdir"] = os.environ["KERNEL_TRACE_DIR"]
        if os.environ.get("KERNEL_TRACE_ALL"):
            tkw["trace_cores"] = list(range(NCORE))
    res = run_bass_kernel_spmd(nc, in_maps, core_ids=list(range(NCORE)), **tkw)
    _CACHE["last_res"] = res
    out = np.concatenate([np.asarray(res.results[i]["out_loc"], np.float32)
                          for i in range(NCORE)], axis=0)
    return out.reshape(B, T, C)
